# revision 35
# baseline (speedup 1.0000x reference)
"""Multi-head causal attention Bass kernel for Trainium2, 8-core SPMD.

Problem: B=2, S=2048, D=1024, H=16, DH=64.
  q = x @ Wq; k = x @ Wk; v = x @ Wv  (per head h: 64-wide column slices)
  out = softmax(causal(q k^T / 8)) v

Sharding: core c -> batch b = c // 4, head group g = c % 4 (heads 4g..4g+3).
Each core gets x[b]^T and 256-wide W column slices (bf16 on host),
computes 4 heads over the full sequence, returns y [2048, 256] f32 in
[seq, (head, dh)] layout; host assembly is a pure column concat.

Per-core pipeline (all matmul operands bf16 - full PE rate at any moving
size, half the DMA/SBUF of f32r; psum f32):
  xT_ch[ch] [128, 8*512]   ch = s-chunk of 512; k-chunk kk at cols 512*kk
  w*_all    [128, 8*256]
  QT/KT[m][ch] [128, 512]  rows = 2 heads x 64 d of pair m, cols = seq
  V4[q]     [128, 4*260]   s-tiles; per head 64 V cols + 1 ones col
                           (ones written once by a strided DVE memset)
  scores^T per step (pair, i-chunk c, j-tile jt): psum [128, 1024], head
      half at col 512*half; diagonal tiles only compute the causally
      valid Ni = 512-o columns -> one ACT exp (scale=1/8) covering both
      heads -> e [128, 2*Ni] bf16; the 128-col diagonal block is masked
      in place by one gpsimd affine_select (keep i >= p, fill 0)
  ctx[i, e] via e-stationary matmuls: out[128 i, 65] += e_slice^T @
      [V_h | ones]; per (pair, head) accumulator psum [128, 4*65] in ONE
      bank (i-tile qq at col 65*qq, col 65*qq+64 = softmax denominator).
      PSUM start/stop groups are bank-granular: only the first matmul
      into the bank starts (zeroing the whole 2KB region), only the last
      stops; causally skipped (i-tile, j-tile) combos are omitted.
  normalize: one strided DVE reciprocal_approx_fast over the 4 l columns
      -> one stride-0-broadcast tensor_tensor multiply -> ctx_sb
      [128, 1024] f32 -> one strided per-pair DMA into y on the SP ring.

Schedule: a flat (pair-major, j-tile) step list with a LOOK=4 software
pipeline - the PE stream is ..., ctx(s-4), scores(s), ... so the
in-order PE queue never reaches a ctx matmul before its exp finished;
projection chains (Q/K/V per chunk, 8 matmul k-chain + DVE copy each)
are paced into the stream by a global deadline scheduler (each chain
lands just before its first consumer, evenly spread otherwise), which
keeps the PE busy through ACT-heavy stretches and hides the exp cost.
Only chunk 0's m0 Q/K chains run before the first attention step.
"""

import sys

import numpy as np

try:
    import concourse.bass as bass  # noqa: F401
except ImportError:
    for _p in ("/opt/trn_rl_repo", "/root/.axon_site/_ro/trn_rl_repo"):
        if _p not in sys.path:
            sys.path.insert(0, _p)
    import concourse.bass as bass  # noqa: F401

from concourse import bacc
import concourse.mybir as mybir
import concourse.tile as tile

F32 = mybir.dt.float32
F32R = mybir.dt.float32r
BF16 = mybir.dt.bfloat16

S = 2048          # sequence length
D = 1024          # model dim (contraction for projections)
HPC = 4           # heads per core
DH = 64           # head dim
NK = D // 128     # 8 contraction chunks
NCH = S // 512    # 4 s-chunks of 512
VW = DH + 1       # 65 cols per head in a V s-tile (V + ones)


def build_kernel(loop_n=0):
    nc = bacc.Bacc("TRN2", target_bir_lowering=False, debug=True)

    # inputs are pre-tiled on the host into the exact SBUF layouts so
    # every input DMA is a flat contiguous per-partition copy
    xT = nc.dram_tensor("xT", [128, NCH * NK * 512], BF16,
                        kind="ExternalInput")
    wq = nc.dram_tensor("wq", [128, NK * HPC * DH], BF16,
                        kind="ExternalInput")
    wk = nc.dram_tensor("wk", [128, NK * HPC * DH], BF16,
                        kind="ExternalInput")
    wv = nc.dram_tensor("wv", [128, NK * HPC * DH], BF16,
                        kind="ExternalInput")
    y = nc.dram_tensor("y", [S, HPC * DH], F32, kind="ExternalOutput")

    with tile.TileContext(nc) as tc:
        from contextlib import ExitStack
        stk = ExitStack()
        loop = stk.enter_context(tc.For_i(0, loop_n, 1)) if loop_n else None
        with stk, (
            tc.tile_pool(name="persist", bufs=1)
        ) as pers, (
            tc.tile_pool(name="proj_ps", bufs=2, space="PSUM")
        ) as proj_ps, (
            tc.tile_pool(name="score_ps", bufs=2, space="PSUM")
        ) as score_ps, (
            tc.tile_pool(name="ctx_ps", bufs=2, space="PSUM")
        ) as ctx_ps_pool, (
            tc.tile_pool(name="esb", bufs=10)
        ) as esb_pool, (
            tc.tile_pool(name="norm", bufs=8)
        ) as norm_pool, (
            tc.tile_pool(name="ctxsb", bufs=3)
        ) as ctxsb_pool:
            # ---- persistent SBUF tiles -------------------------------------
            xT_ch = [
                pers.tile([128, NK * 512], BF16, tag=f"xTc{ch}", name=f"xTc{ch}")
                for ch in range(NCH)
            ]
            w_all = {
                wname: pers.tile([128, NK * HPC * DH], BF16,
                                 name=f"w_{wname}")
                for wname in ("q", "k", "v")
            }
            QT_sb = [
                [pers.tile([128, 512], BF16, tag=f"QT{m}c{ch}",
                           name=f"QT{m}c{ch}") for ch in range(NCH)]
                for m in range(2)
            ]
            KT_sb = [
                [pers.tile([128, 512], BF16, tag=f"KT{m}c{ch}",
                           name=f"KT{m}c{ch}") for ch in range(NCH)]
                for m in range(2)
            ]
            V4 = [
                pers.tile([128, 4 * HPC * VW], BF16, tag=f"V4_{q}",
                          name=f"V4_{q}")
                for q in range(4)
            ]

            def xs(ch, kk):      # xT chunk ch, k-chunk kk -> [128, 512]
                return xT_ch[ch][:, 512 * kk:512 * (kk + 1)]

            def ws(wname, kk):   # w k-chunk [128, 256]
                return w_all[wname][:, HPC * DH * kk:HPC * DH * (kk + 1)]

            def vs(t):           # V s-tile t -> [128, 260]
                q, r = t // 4, t % 4
                return V4[q][:, HPC * VW * r:HPC * VW * (r + 1)]

            # ---- input DMAs ------------------------------------------------
            # Chunk 0 of x gates the first projection chain: split it
            # across both HWDGE rings (half on ACT, half on SP ahead of
            # the W loads) so it lands in roughly half the time.
            # DMA bandwidth is shared: only w_q (needed by the first
            # chain) rides the SP ring next to x0; w_k/w_v queue on the
            # ACT ring behind x0 so they don't halve x0's bandwidth.
            CW = NK * 512  # per-chunk flat width
            # w_q and w_k gate the first two projection chains: both ride
            # the SP ring while x0 streams on the ACT ring; w_v queues
            # behind x0 (its V chains run later)
            nc.sync.dma_start(out=w_all["q"][:], in_=wq[:])
            nc.sync.dma_start(out=w_all["k"][:], in_=wk[:])
            # three pieces: the first projection chains consume k-chunks
            # as they land instead of waiting for the whole chunk
            for a, b in ((0, 3), (3, 6), (6, 8)):
                nc.scalar.dma_start(
                    out=xT_ch[0][:, 512 * a:512 * b],
                    in_=xT[:, 512 * a:512 * b])
            nc.scalar.dma_start(out=w_all["v"][:], in_=wv[:])
            for ch in range(1, NCH):
                nc.scalar.dma_start(
                    out=xT_ch[ch][:],
                    in_=xT[:, CW * ch:CW * (ch + 1)],
                )
            # ones columns of V (softmax denominator): one strided memset
            # per V4 group
            for q in range(4):
                nc.vector.memset(
                    V4[q].rearrange("p (t h c) -> p t h c", t=4, h=HPC)[
                        :, :, :, DH:DH + 1
                    ],
                    1.0,
                )

            # ---- projection pieces -----------------------------------------
            def emit_qk(ch, m, wname, dest):
                # Q^T/K^T: out[j, s] = sum_d W[d, j] * xT[d, s]
                ps = proj_ps.tile([128, 512], F32, tag="proj", name="ps_qk")
                for kk in range(NK):
                    nc.tensor.matmul(
                        ps[:],
                        ws(wname, kk)[:, 128 * m:128 * (m + 1)],
                        xs(ch, kk),
                        start=(kk == 0),
                        stop=(kk == NK - 1),
                    )
                nc.vector.tensor_copy(dest[m][ch][:], ps[:])

            def emit_v(t):
                # V: out[s, e] = sum_d xT[d, s] * Wv[d, e]
                ps = proj_ps.tile([128, HPC * DH], F32, tag="proj", name="ps_v")
                for kk in range(NK):
                    nc.tensor.matmul(
                        ps[:],
                        xs(t // 4, kk)[:, 128 * (t % 4):128 * (t % 4 + 1)],
                        ws("v", kk),
                        start=(kk == 0),
                        stop=(kk == NK - 1),
                    )
                nc.vector.tensor_copy(
                    vs(t).rearrange("p (h c) -> p h c", h=HPC)[:, :, 0:DH],
                    ps.rearrange("p (h c) -> p h c", h=HPC),
                )

            def proj_pieces(ch):
                pieces = []
                for m in range(2):
                    for wname, dest in (("q", QT_sb), ("k", KT_sb)):
                        pieces.append(
                            lambda ch=ch, m=m, w=wname, d=dest: emit_qk(ch, m, w, d))
                for t in range(4 * ch, 4 * ch + 4):
                    pieces.append(lambda t=t: emit_v(t))
                return pieces

            # Interleave plan: while attention chunk c runs, emit the
            # projection pieces listed here, spaced over c's steps.
            # Chunk 3's K/V projections are only needed from j-tile 12 on,
            # so they slide into attention chunk 3 itself; its Q must be
            # ready at step 0 and is emitted during chunk 2.
            # Global deadline-paced projection schedule. Front index of
            # chunk c starts at FSTART[c]; a piece must be emitted at a
            # front index <= its deadline (one before its first consumer
            # in the in-order PE stream).
            LOOK = 4
            FSTART = [0, 8, 24, 48, 80]
            sched = []  # (deadline, piece)
            for ch in range(NCH):
                pc = proj_pieces(ch)  # [q m0, k m0, q m1, k m1, v*4]
                F, njt = FSTART[ch], 4 * (ch + 1)
                if ch == 0:
                    q_dl = {0: -1, 1: 3}   # emitted pre-loop / before pair1
                    k_dl = {0: -1, 1: 3}
                else:
                    q_dl = {0: F - 1, 1: F + njt - 1}
                    k_dl = {0: F + 4 * ch - 1, 1: F + njt + 4 * ch - 1}
                sched.append((q_dl[0], pc[0]))
                sched.append((k_dl[0], pc[1]))
                sched.append((q_dl[1], pc[2]))
                sched.append((k_dl[1], pc[3]))
                for i, t in enumerate(range(4 * ch, 4 * ch + 4)):
                    # V_t first consumed by the back of j-tile t, which
                    # runs in iteration F + t + LOOK (after that
                    # iteration's front and pieces)
                    sched.append((F + t + LOOK - 1, pc[4 + i]))
            sched.sort(key=lambda d: d[0])
            pre = [p for dl, p in sched if dl < 0]
            sched = [d for d in sched if d[0] >= 0]

            # ---- attention -------------------------------------------------
            def attn_step(c, pair, jt, ctx_ab):
                m = pair
                o = 128 * (jt - 4 * c) if jt >= 4 * c else 0
                ni = 512 - o
                eo = o  # first computed i_rel column
                ps = score_ps.tile([128, 1024], F32, tag="score", name="s_ps")
                for half in range(2):
                    off = half * 64
                    nc.tensor.matmul(
                        ps[:, 512 * half:512 * half + ni],
                        KT_sb[m][jt // 4][off:off + 64,
                                          128 * (jt % 4):128 * (jt % 4 + 1)],
                        QT_sb[m][c][off:off + 64, eo:512],
                        start=True,
                        stop=True,
                        tile_position=(off, 0),
                    )
                e = esb_pool.tile([128, 2 * ni], BF16, tag="esb", name="e_sb")
                e3 = e.rearrange("p (h i) -> p h i", h=2)
                ps3 = ps.rearrange("p (h i) -> p h i", h=2)[:, :, 0:ni]
                nc.scalar.activation(
                    out=e3, in_=ps3,
                    func=mybir.ActivationFunctionType.Exp, scale=0.125,
                )
                if jt >= 4 * c:
                    # diagonal block: keep where i - p >= 0 within the
                    # 128 columns at the diagonal
                    d0 = o - eo
                    nc.gpsimd.affine_select(
                        e3[:, :, d0:d0 + 128],
                        e3[:, :, d0:d0 + 128],
                        pattern=[[0, 2], [1, 128]],
                        compare_op=mybir.AluOpType.is_ge,
                        fill=0.0,
                        base=0,
                        channel_multiplier=-1,
                    )
                return e3

            def attn_back(c, pair, jt, e3, ctx_ab):
                o = 128 * (jt - 4 * c) if jt >= 4 * c else 0
                eo = o
                # one psum accumulation group per bank: start zeroes the
                # whole 2KB zero-region, so only the first matmul into the
                # bank starts, later i-tile regions overwrite-on-first-touch
                for qq in range(4):
                    qg = 4 * c + qq
                    if jt > qg:
                        continue
                    i0 = 128 * qq - eo
                    for head in range(2):
                        nc.tensor.matmul(
                            ctx_ab[head][:, VW * qq:VW * (qq + 1)],
                            e3[:, head, i0:i0 + 128],
                            vs(jt)[:, VW * (2 * pair + head):
                                   VW * (2 * pair + head + 1)],
                            start=(jt == 0 and qq == 0),
                            stop=(jt == 4 * c + 3 and qq == 3),
                        )

            def normalize(c, pair, head, ctx_psum, ctx_sb_c):
                h = 2 * pair + head
                recip = norm_pool.tile([128, 4], F32, tag="recip", name="recip")
                nc.vector.reciprocal_approx_fast(
                    out=recip[:],
                    in_=ctx_psum.rearrange("p (q e) -> p q e", q=4)[
                        :, :, DH:DH + 1],
                )
                # one broadcast multiply for all four i-tiles: recip
                # [128, 4, 1] stride-0-expanded along the 64 e-columns
                ctx_v = ctx_psum.rearrange("p (q e) -> p q e", q=4)[
                    :, :, 0:DH]
                out_v = ctx_sb_c.rearrange("p (q e) -> p q e", q=4)[
                    :, :, 64 * h:64 * (h + 1)]
                rec_v = recip.rearrange("p (q x) -> p q x", x=1)
                rec_b, _ = bass.broadcast_tensor_aps(rec_v, ctx_v)
                nc.vector.tensor_mul(out_v, ctx_v, rec_b)

            # only the two chains the first attention step needs run
            # before the flat schedule; everything else interleaves
            for piece in pre:
                piece()

            # Flat schedule with a two-step lookahead: the PE stream is
            # ..., ctx(s-2), scores(s), proj filler, ... so by the time
            # the in-order PE queue reaches a ctx matmul, its exp (ACT)
            # finished two steps ago and the engine never stalls on it.
            all_steps = [
                (c, pair, jt)
                for c in range(NCH)
                for pair in range(2)
                for jt in range(4 * (c + 1))
            ]
            e3s = {}
            ctx_ab = {}
            ctx_sb_by_c = {}

            def do_back(step):
                c, pair, jt = step
                njt = 4 * (c + 1)
                if jt == 0:
                    if pair == 0:
                        ctx_sb_by_c[c] = ctxsb_pool.tile(
                            [128, 4 * HPC * DH], F32, tag="ctxsb",
                            name="ctx_sb")
                    ctx_ab[(c, pair)] = [
                        ctx_ps_pool.tile([128, 4 * VW], F32, tag="ctx",
                                         name=f"ctx_ps{head}")
                        for head in range(2)
                    ]
                attn_back(c, pair, jt, e3s.pop(step), ctx_ab[(c, pair)])
                # psum reads must wait for the bank's accumulation group
                # to stop (last j-tile of the pair)
                if jt == njt - 1:
                    for head in range(2):
                        normalize(c, pair, head, ctx_ab[(c, pair)][head],
                                  ctx_sb_by_c[c])
                    # one strided output DMA per pair, on the SP ring
                    # (idle mid-iteration; W reload next iteration only
                    # trails the last of these by ~1 us)
                    nc.sync.dma_start(
                        out=y[512 * c:512 * (c + 1),
                              128 * pair:128 * (pair + 1)].rearrange(
                            "(q p) e -> p q e", q=4),
                        in_=ctx_sb_by_c[c].rearrange(
                            "p (q x) -> p q x", q=4)[
                            :, :, 128 * pair:128 * (pair + 1)],
                    )

            n_pieces_total = len(sched)
            front_count = [0]
            pieces_done = [0]

            def do_front(step):
                c, pair, jt = step
                e3s[step] = attn_step(c, pair, jt, None)
                idx = front_count[0]
                front_count[0] += 1
                # deadline-constrained even pacing of projection chains
                while sched and (
                    sched[0][0] <= idx
                    or pieces_done[0] * 80 <= idx * n_pieces_total
                ):
                    sched.pop(0)[1]()
                    pieces_done[0] += 1

            n_all = len(all_steps)
            for k in range(n_all + LOOK):
                if k >= LOOK:
                    do_back(all_steps[k - LOOK])
                if k < n_all:
                    do_front(all_steps[k])
    nc.compile()
    return nc


_CACHED = None


def get_nc():
    global _CACHED
    if _CACHED is None:
        _CACHED = build_kernel()
    return _CACHED


def shard_inputs(x, W_query, W_key, W_value):
    """Full inputs -> per-core input maps (bf16 on host)."""
    bf16 = mybir.dt.np(BF16)
    in_maps = []

    def tile_x(xb):
        # x[b].T [1024, 2048] -> [p, chunk, kk, s] partition-major flat
        xt = np.ascontiguousarray(xb.T).reshape(NK, 128, NCH, 512)
        return np.ascontiguousarray(
            xt.transpose(1, 2, 0, 3).reshape(128, NCH * NK * 512)
        ).astype(bf16)

    def tile_w(w):
        # W column slice [1024, 256] -> [p, kk, e] partition-major flat
        return np.ascontiguousarray(
            w.reshape(NK, 128, HPC * DH).transpose(1, 0, 2).reshape(
                128, NK * HPC * DH)
        ).astype(bf16)

    # one retile per batch, shared by the 4 cores of that batch
    xT_by_batch = [tile_x(x[b]) for b in range(2)]
    for core in range(8):
        b, g = core // 4, core % 4
        sl = slice(256 * g, 256 * (g + 1))
        in_maps.append({
            "xT": xT_by_batch[b],
            "wq": tile_w(W_query[:, sl]),
            "wk": tile_w(W_key[:, sl]),
            "wv": tile_w(W_value[:, sl]),
        })
    return in_maps


def assemble_output(results):
    """Per-core y [S, 256] -> full [2, S, 1024]."""
    out = np.empty((2, S, 1024), np.float32)
    for core in range(8):
        b, g = core // 4, core % 4
        out[b, :, 256 * g:256 * (g + 1)] = results[core]["y"]
    return out


def kernel(x, W_query, W_key, W_value):
    """Full inputs in, full output out; 8-core SPMD underneath."""
    from concourse.bass_utils import run_bass_kernel_spmd

    x = np.ascontiguousarray(np.asarray(x, dtype=np.float32))
    W_query = np.ascontiguousarray(np.asarray(W_query, dtype=np.float32))
    W_key = np.ascontiguousarray(np.asarray(W_key, dtype=np.float32))
    W_value = np.ascontiguousarray(np.asarray(W_value, dtype=np.float32))

    nc = get_nc()
    in_maps = shard_inputs(x, W_query, W_key, W_value)
    last_err = None
    for _attempt in range(3):
        try:
            res = run_bass_kernel_spmd(nc, in_maps, core_ids=list(range(8)))
            return assemble_output(res.results)
        except Exception as e:  # transient device wedges seen on this fabric
            last_err = e
            import time as _time
            _time.sleep(2.0)
    raise last_err
